# revision 7
# baseline (speedup 1.0000x reference)
"""DecodeNextToken Bass kernel for 8 trn2 NeuronCores. Self-contained."""
import numpy as np

B, V, T = 512, 50257, 2048
NC = 8
RB = B // NC            # 64 rows per core
VP = 50304              # padded vocab
H = VP // 2             # 25152 (half per partition-block)
CH = H // 8             # 3144
SUB = 786               # candidate subchunk (32 per half)
NEG = np.float32(-1e30)

_cache = {}


def _build(temp, top_p, rp):
    import concourse.bacc as bacc
    import concourse.bass as bass
    import concourse.mybir as mybir
    from concourse.tile import TileContext

    f32 = mybir.dt.float32
    i32 = mybir.dt.int32
    u8 = mybir.dt.uint8
    u32 = mybir.dt.uint32
    AF = mybir.ActivationFunctionType
    OP = mybir.AluOpType
    X = mybir.AxisListType.X

    nc = bacc.Bacc("TRN2", target_bir_lowering=False, debug=False,
                   enable_asserts=False, num_devices=NC)
    lg_d = nc.dram_tensor("lg", [RB, VP], f32, kind="ExternalInput").ap()
    ru_d = nc.dram_tensor("ru", [RB, VP], f32, kind="ExternalInput").ap()
    pv_d = nc.dram_tensor("pv", [128, T // 2], i32, kind="ExternalInput").ap()
    rb_d = nc.dram_tensor("rb", [128, 1], i32, kind="ExternalInput").ap()
    io_d = nc.dram_tensor("io", [64, 64], f32, kind="ExternalInput").ap()
    cb_d = nc.dram_tensor("cb", [128, 8], f32, kind="ExternalInput").ap()
    probs_d = nc.dram_tensor("probs", [RB, V], f32, kind="ExternalOutput").ap()
    idx_d = nc.dram_tensor("idx", [RB, 1], i32, kind="ExternalOutput").ap()
    mask_d = nc.dram_tensor("maskb", [RB, VP], u8, kind="ExternalOutput").ap()
    s1_d = nc.dram_tensor("s1", [128, 8], f32, kind="Internal").ap()
    s2_d = nc.dram_tensor("s2", [128, 256], f32, kind="Internal").ap()

    with TileContext(nc) as tc, nc.allow_non_contiguous_dma(reason="tiny roundtrips"):
        A = lambda nm, sh, dt: nc.alloc_sbuf_tensor(nm, sh, dt).ap()
        x = A("x", [128, H], f32)
        msk = A("msk", [128, H], u8)
        pv = A("pvs", [128, T // 2], i32)
        rbs = A("rbs", [128, 1], i32)
        offs = A("offs", [128, T // 2], i32)
        ones = A("ones", [128, 1], u8)
        tA = A("tA", [128, CH], f32)
        rr = tA
        e = A("e", [128, CH], f32)
        tB = e
        o = A("o", [128, CH], f32)
        ru = A("rusb", [128, CH], f32)
        zp = A("zp", [128, 8], f32)
        cand = A("cand", [128, 256], f32)
        mc = A("mc", [128, 64], f32)
        mi = A("mi", [128, 64], u32)
        sm8 = A("sm8", [128, 8], f32)
        mif = A("mif", [128, 8], f32)
        cb8 = A("cb8", [128, 8], f32)
        ohc = A("ohc", [128, 8], f32)
        pos = A("pos", [128, 8], f32)
        posw = A("posw", [128, 1], f32)
        rm = A("rm", [128, 1], f32)
        negm128 = A("negm128", [128, 1], f32)
        thr128 = A("thr128", [128, 1], f32)
        b128 = A("b128", [128, 1], f32)
        # per-row (partitions 0..63) tiles
        h2 = A("h2", [64, 2], f32)
        m64 = A("m64", [64, 1], f32)
        negm64 = A("negm64", [64, 1], f32)
        negm64t = A("negm64t", [64, 1], f32)
        z2 = A("z2", [64, 2], f32)
        z64 = A("z64", [64, 1], f32)
        thrz = A("thrz", [64, 1], f32)
        candr = A("candr", [64, 512], f32)
        wk = A("wk", [64, 512], f32)
        srt = A("srt", [64, 64], f32)
        iot = A("iot", [64, 64], f32)
        e1 = A("e1", [64, 64], f32)
        Ssc = A("Ssc", [64, 64], f32)
        cle = A("cle", [64, 64], f32)
        n1 = A("n1", [64, 1], f32)
        Jc = A("Jc", [64, 1], f32)
        Jm1 = A("Jm1", [64, 1], f32)
        oh = A("oh", [64, 64], f32)
        tmp = A("tmp", [64, 64], f32)
        thr64 = A("thr64", [64, 1], f32)
        e2 = A("e2", [64, 64], f32)
        km = A("km", [64, 64], f32)
        zj = A("zj", [64, 1], f32)
        lnzj = A("lnzj", [64, 1], f32)
        b64 = A("b64", [64, 1], f32)
        wp2 = A("wp2", [64, 2], f32)
        wv2 = A("wv2", [64, 2], f32)
        sel = A("sel", [64, 1], f32)
        d1 = A("d1", [64, 1], f32)
        idxf = A("idxf", [64, 1], f32)
        idxi = A("idxi", [64, 1], i32)

        # ---- load logits: block layout p<64 -> (row p, half0), p>=64 -> (row p-64, half1)
        nc.sync.dma_start(out=x[0:64, :], in_=lg_d[:, 0:H])
        nc.sync.dma_start(out=x[64:128, :], in_=lg_d[:, H:VP])
        nc.sync.dma_start(out=iot, in_=io_d)
        nc.sync.dma_start(out=cb8, in_=cb_d)
        # ---- membership scatter into pre-zeroed DRAM mask
        nc.sync.dma_start(out=pv, in_=pv_d)
        nc.sync.dma_start(out=rbs, in_=rb_d)
        nc.vector.memset(ones, 1)
        nc.vector.tensor_tensor(out=offs, in0=pv, in1=rbs.to_broadcast([128, T // 2]),
                                op=OP.add)
        for k in range(T // 2):
            nc.gpsimd.indirect_dma_start(
                out=mask_d, out_offset=bass.IndirectOffsetOnAxis(ap=offs[:, k:k + 1], axis=1),
                in_=ones, in_offset=None,
                bounds_check=RB * VP - 1, oob_is_err=False)
        nc.sync.dma_start(out=msk[0:64, :], in_=mask_d[:, 0:H])
        nc.sync.dma_start(out=msk[64:128, :], in_=mask_d[:, H:VP])
        # ---- penalty: x = mask ? min(x*rp, x/rp) : x   (chunked)
        for c in range(8):
            xc = x[:, c * CH:(c + 1) * CH]
            nc.scalar.mul(out=tA, in_=xc, mul=float(1.0 / rp))
            nc.vector.scalar_tensor_tensor(out=tB, in0=xc, scalar=float(rp),
                                           in1=tA, op0=OP.mult, op1=OP.min)
            nc.vector.copy_predicated(out=xc, mask=msk[:, c * CH:(c + 1) * CH], data=tB)
        # ---- row max (chunked)
        for c in range(8):
            nc.vector.tensor_reduce(out=zp[:, c:c + 1], in_=x[:, c * CH:(c + 1) * CH],
                                    axis=X, op=OP.max)
        nc.vector.tensor_reduce(out=rm, in_=zp, axis=X, op=OP.max)
        nc.sync.dma_start(out=s1_d[:, 0:1], in_=rm)
        nc.sync.dma_start(out=h2[:, 0:1], in_=s1_d[0:64, 0:1])
        nc.sync.dma_start(out=h2[:, 1:2], in_=s1_d[64:128, 0:1])
        nc.vector.tensor_reduce(out=m64, in_=h2, axis=X, op=OP.max)
        nc.vector.tensor_scalar_mul(negm64, m64, -1.0)
        nc.sync.dma_start(out=s1_d[0:64, 1:2], in_=negm64)
        nc.sync.dma_start(out=negm128[0:64, :], in_=s1_d[0:64, 1:2])
        nc.sync.dma_start(out=negm128[64:128, :], in_=s1_d[0:64, 1:2])
        # ---- Z = sum exp(x - m)
        for c in range(8):
            nc.scalar.activation(out=e, in_=x[:, c * CH:(c + 1) * CH], func=AF.Exp,
                                 bias=negm128, scale=1.0, accum_out=zp[:, c:c + 1])
        nc.vector.tensor_reduce(out=rm, in_=zp, axis=X, op=OP.add)
        nc.sync.dma_start(out=s1_d[:, 2:3], in_=rm)
        nc.sync.dma_start(out=z2[:, 0:1], in_=s1_d[0:64, 2:3])
        nc.sync.dma_start(out=z2[:, 1:2], in_=s1_d[64:128, 2:3])
        nc.vector.tensor_reduce(out=z64, in_=z2, axis=X, op=OP.add)
        nc.vector.tensor_scalar_mul(thrz, z64, float(top_p))
        # ---- candidates: top-8 per 786-subchunk
        for s in range(32):
            nc.vector.max(out=cand[:, 8 * s:8 * s + 8], in_=x[:, SUB * s:SUB * (s + 1)])
        nc.sync.dma_start(out=s2_d, in_=cand)
        nc.sync.dma_start(out=candr[:, 0:256], in_=s2_d[0:64, :])
        nc.sync.dma_start(out=candr[:, 256:512], in_=s2_d[64:128, :])
        # ---- sort top-56 of the 512 candidates (values only)
        nc.vector.tensor_copy(out=wk, in_=candr)
        for j in range(7):
            nc.vector.max(out=srt[:, 8 * j:8 * j + 8], in_=wk)
            nc.vector.match_replace(out=wk, in_to_replace=srt[:, 8 * j:8 * j + 8],
                                    in_values=wk, imm_value=-3e38)
        # ---- per-row small math
        nc.scalar.activation(out=e1[:, 0:56], in_=srt[:, 0:56], func=AF.Exp,
                             bias=negm64, scale=1.0)
        nc.vector.tensor_tensor_scan(out=Ssc[:, 0:56], data0=e1[:, 0:56], data1=e1[:, 0:56],
                                     initial=0.0, op0=OP.add, op1=OP.bypass)
        nc.vector.tensor_tensor(out=cle, in0=Ssc, in1=thrz.to_broadcast([64, 64]), op=OP.is_le)
        nc.vector.tensor_reduce(out=n1, in_=cle[:, 1:51], axis=X, op=OP.add)
        nc.vector.tensor_scalar(Jc, n1, 1.0, 50.0, op0=OP.add, op1=OP.min)
        nc.vector.tensor_scalar_add(Jm1, Jc, -1.0)
        nc.vector.tensor_tensor(out=oh, in0=iot, in1=Jm1.to_broadcast([64, 64]), op=OP.is_equal)
        nc.vector.tensor_tensor(out=tmp[:, 0:56], in0=srt[:, 0:56], in1=oh[:, 0:56], op=OP.mult)
        nc.vector.tensor_reduce(out=thr64, in_=tmp[:, 0:56], axis=X, op=OP.add)
        nc.vector.tensor_scalar_mul(negm64t, negm64, float(1.0 / temp))
        nc.scalar.activation(out=e2[:, 0:56], in_=srt[:, 0:56], func=AF.Exp,
                             bias=negm64t, scale=float(1.0 / temp))
        nc.vector.tensor_tensor(out=km, in0=iot, in1=Jc.to_broadcast([64, 64]), op=OP.is_lt)
        nc.vector.tensor_tensor(out=tmp[:, 0:56], in0=e2[:, 0:56], in1=km[:, 0:56], op=OP.mult)
        nc.vector.tensor_reduce(out=zj, in_=tmp[:, 0:56], axis=X, op=OP.add)
        nc.scalar.activation(out=lnzj, in_=zj, func=AF.Ln)
        nc.vector.tensor_scalar_mul(lnzj, lnzj, -1.0)
        nc.vector.tensor_tensor(out=b64, in0=lnzj, in1=negm64t, op=OP.add)
        nc.sync.dma_start(out=s1_d[0:64, 3:4], in_=thr64)
        nc.sync.dma_start(out=s1_d[0:64, 4:5], in_=b64)
        nc.sync.dma_start(out=thr128[0:64, :], in_=s1_d[0:64, 3:4])
        nc.sync.dma_start(out=thr128[64:128, :], in_=s1_d[0:64, 3:4])
        nc.sync.dma_start(out=b128[0:64, :], in_=s1_d[0:64, 4:5])
        nc.sync.dma_start(out=b128[64:128, :], in_=s1_d[0:64, 4:5])
        # ---- probs + gumbel argmax, chunked
        for c in range(8):
            xc = x[:, c * CH:(c + 1) * CH]
            nc.scalar.activation(out=e, in_=xc, func=AF.Exp, bias=b128,
                                 scale=float(1.0 / temp))
            nc.vector.tensor_tensor(out=tA, in0=xc, in1=thr128.to_broadcast([128, CH]),
                                    op=OP.is_ge)
            nc.vector.tensor_tensor(out=o, in0=tA, in1=e, op=OP.mult)
            nc.sync.dma_start(out=probs_d[:, c * CH:(c + 1) * CH], in_=o[0:64, :])
            if c < 7:
                nc.sync.dma_start(out=probs_d[:, H + c * CH:H + (c + 1) * CH],
                                  in_=o[64:128, :])
            else:
                nc.sync.dma_start(out=probs_d[:, H + c * CH:V],
                                  in_=o[64:128, 0:V - H - 7 * CH])
            nc.sync.dma_start(out=ru[0:64, :], in_=ru_d[:, c * CH:(c + 1) * CH])
            nc.sync.dma_start(out=ru[64:128, :], in_=ru_d[:, H + c * CH:H + (c + 1) * CH])
            nc.scalar.activation(out=rr, in_=ru, func=AF.Ln)
            nc.vector.reciprocal(out=ru, in_=rr)
            nc.vector.scalar_tensor_tensor(out=rr, in0=o, scalar=-1.0, in1=ru,
                                           op0=OP.mult, op1=OP.mult)
            nc.vector.max(out=mc[:, 8 * c:8 * c + 8], in_=rr)
            nc.vector.max_index(out=mi[:, 8 * c:8 * c + 8], in_max=mc[:, 8 * c:8 * c + 8],
                                in_values=rr)
        # ---- winner per partition, then per row
        nc.vector.max(out=sm8, in_=mc)
        for c in range(8):
            nc.vector.tensor_copy(out=pos[:, c:c + 1], in_=mc[:, 8 * c:8 * c + 1])
            nc.vector.tensor_copy(out=mif[:, c:c + 1], in_=mi[:, 8 * c:8 * c + 1])
        nc.vector.tensor_tensor(out=ohc, in0=pos,
                                in1=sm8[:, 0:1].to_broadcast([128, 8]), op=OP.is_equal)
        nc.vector.tensor_tensor(out=pos, in0=mif, in1=cb8, op=OP.add)
        nc.vector.tensor_tensor(out=pos, in0=pos, in1=ohc, op=OP.mult)
        nc.vector.tensor_reduce(out=posw, in_=pos, axis=X, op=OP.add)
        nc.sync.dma_start(out=s1_d[:, 5:6], in_=posw)
        nc.sync.dma_start(out=s1_d[:, 6:7], in_=sm8[:, 0:1])
        nc.sync.dma_start(out=wp2[:, 0:1], in_=s1_d[0:64, 5:6])
        nc.sync.dma_start(out=wp2[:, 1:2], in_=s1_d[64:128, 5:6])
        nc.sync.dma_start(out=wv2[:, 0:1], in_=s1_d[0:64, 6:7])
        nc.sync.dma_start(out=wv2[:, 1:2], in_=s1_d[64:128, 6:7])
        nc.vector.tensor_tensor(out=sel, in0=wv2[:, 0:1], in1=wv2[:, 1:2], op=OP.is_ge)
        nc.vector.tensor_tensor(out=d1, in0=wp2[:, 0:1], in1=wp2[:, 1:2], op=OP.subtract)
        nc.vector.tensor_scalar_add(d1, d1, float(-H))
        nc.vector.tensor_tensor(out=d1, in0=d1, in1=sel, op=OP.mult)
        nc.vector.tensor_tensor(out=idxf, in0=d1, in1=wp2[:, 1:2], op=OP.add)
        nc.vector.tensor_scalar_add(idxf, idxf, float(H))
        nc.vector.tensor_copy(out=idxi, in_=idxf)
        nc.sync.dma_start(out=idx_d, in_=idxi)

    nc.compile()
    return nc


def _get(temp, top_p, rp):
    key = (round(float(temp), 6), round(float(top_p), 6), round(float(rp), 6))
    if key not in _cache:
        _cache[key] = _build(float(temp), float(top_p), float(rp))
    return _cache[key]


def kernel(logits, previous_tokens, rand_u, temperature, top_p, repetition_penalty):
    from concourse import bass_utils
    logits = np.asarray(logits, np.float32)
    prev = np.asarray(previous_tokens)
    randu = np.asarray(rand_u, np.float32)
    temp = float(np.asarray(temperature).ravel()[0])
    tp = float(np.asarray(top_p).ravel()[0])
    rp = float(np.asarray(repetition_penalty).ravel()[0])
    nc = _get(temp, tp, rp)

    lg = np.full((B, VP), NEG, np.float32)
    lg[:, :V] = logits
    ru = np.full((B, VP), 0.5, np.float32)
    ru[:, :V] = randu
    pv128 = prev.astype(np.int32).reshape(B // RB, RB, 2, T // 2).transpose(0, 2, 1, 3)
    rb = (np.arange(128, dtype=np.int32) % 64 * VP).reshape(128, 1)
    io = np.broadcast_to(np.arange(64, dtype=np.float32), (64, 64)).copy()
    cb = np.broadcast_to((np.arange(8, dtype=np.float32) * CH), (128, 8)).copy()

    in_maps = []
    for c in range(NC):
        in_maps.append({
            "lg": lg[c * RB:(c + 1) * RB],
            "ru": ru[c * RB:(c + 1) * RB],
            "pv": np.ascontiguousarray(pv128[c].reshape(128, T // 2)),
            "rb": rb, "io": io, "cb": cb,
        })
    res = bass_utils.run_bass_kernel_spmd(nc, in_maps, core_ids=list(range(NC)))
    probs = np.concatenate([res.results[c]["probs"] for c in range(NC)], axis=0)
    idx = np.concatenate([res.results[c]["idx"] for c in range(NC)], axis=0)
    out_dtype = np.int64 if prev.dtype == np.int64 else np.int32
    return idx.astype(out_dtype), probs


# revision 9
# speedup vs baseline: 1.4037x; 1.4037x over previous
"""DecodeNextToken Bass kernel for 8 trn2 NeuronCores. Self-contained."""
import numpy as np

B, V, T = 512, 50257, 2048
NC = 8
RB = B // NC            # 64 rows per core
VP = 50304              # padded vocab
H = VP // 2             # 25152 (half per partition-block)
CH = H // 8             # 3144
SUB = 786               # candidate subchunk (32 per half)
NEG = np.float32(-1e30)

_cache = {}


def _build(temp, top_p, rp):
    import concourse.bacc as bacc
    import concourse.bass as bass
    import concourse.mybir as mybir
    from concourse.tile import TileContext

    f32 = mybir.dt.float32
    i32 = mybir.dt.int32
    u8 = mybir.dt.uint8
    u32 = mybir.dt.uint32
    AF = mybir.ActivationFunctionType
    OP = mybir.AluOpType
    X = mybir.AxisListType.X

    nc = bacc.Bacc("TRN2", target_bir_lowering=False, debug=False,
                   enable_asserts=False, num_devices=NC)
    lg_d = nc.dram_tensor("lg", [RB, VP], f32, kind="ExternalInput").ap()
    ru_d = nc.dram_tensor("ru", [RB, VP], f32, kind="ExternalInput").ap()
    pv_d = nc.dram_tensor("pv", [128, T // 2], i32, kind="ExternalInput").ap()
    rb_d = nc.dram_tensor("rb", [128, 1], i32, kind="ExternalInput").ap()
    io_d = nc.dram_tensor("io", [64, 64], f32, kind="ExternalInput").ap()
    cb_d = nc.dram_tensor("cb", [128, 8], f32, kind="ExternalInput").ap()
    probs_d = nc.dram_tensor("probs", [RB, V], f32, kind="ExternalOutput").ap()
    idx_d = nc.dram_tensor("idx", [RB, 1], i32, kind="ExternalOutput").ap()
    mask_d = nc.dram_tensor("maskb", [RB, VP], u8, kind="ExternalOutput").ap()
    s1_d = nc.dram_tensor("s1", [128, 8], f32, kind="Internal").ap()
    s2_d = nc.dram_tensor("s2", [128, 256], f32, kind="Internal").ap()

    with TileContext(nc) as tc, nc.allow_non_contiguous_dma(reason="tiny roundtrips"):
        A = lambda nm, sh, dt: nc.alloc_sbuf_tensor(nm, sh, dt).ap()
        x = A("x", [128, H], f32)
        msk = A("msk", [128, H], u8)
        pv = A("pvs", [128, T // 2], i32)
        rbs = A("rbs", [128, 1], i32)
        offs = A("offs", [128, T // 2], i32)
        ones = A("ones", [128, 1], u8)
        tA = A("tA", [128, CH], f32)
        rr = tA
        e = A("e", [128, CH], f32)
        tB = e
        o = A("o", [128, CH], f32)
        ru = A("rusb", [128, CH], f32)
        zp = A("zp", [128, 8], f32)
        cand = A("cand", [128, 256], f32)
        mc = A("mc", [128, 64], f32)
        mi = A("mi", [128, 64], u32)
        sm8 = A("sm8", [128, 8], f32)
        mif = A("mif", [128, 8], f32)
        cb8 = A("cb8", [128, 8], f32)
        ohc = A("ohc", [128, 8], f32)
        pos = A("pos", [128, 8], f32)
        posw = A("posw", [128, 1], f32)
        rm = A("rm", [128, 1], f32)
        negm128 = A("negm128", [128, 1], f32)
        thr128 = A("thr128", [128, 1], f32)
        b128 = A("b128", [128, 1], f32)
        # per-row (partitions 0..63) tiles
        h2 = A("h2", [64, 2], f32)
        m64 = A("m64", [64, 1], f32)
        negm64 = A("negm64", [64, 1], f32)
        negm64t = A("negm64t", [64, 1], f32)
        z2 = A("z2", [64, 2], f32)
        z64 = A("z64", [64, 1], f32)
        thrz = A("thrz", [64, 1], f32)
        candr = A("candr", [64, 512], f32)
        wk = A("wk", [64, 512], f32)
        srt = A("srt", [64, 64], f32)
        iot = A("iot", [64, 64], f32)
        e1 = A("e1", [64, 64], f32)
        Ssc = A("Ssc", [64, 64], f32)
        cle = A("cle", [64, 64], f32)
        n1 = A("n1", [64, 1], f32)
        Jc = A("Jc", [64, 1], f32)
        Jm1 = A("Jm1", [64, 1], f32)
        oh = A("oh", [64, 64], f32)
        tmp = A("tmp", [64, 64], f32)
        thr64 = A("thr64", [64, 1], f32)
        e2 = A("e2", [64, 64], f32)
        km = A("km", [64, 64], f32)
        zj = A("zj", [64, 1], f32)
        lnzj = A("lnzj", [64, 1], f32)
        b64 = A("b64", [64, 1], f32)
        wp2 = A("wp2", [64, 2], f32)
        wv2 = A("wv2", [64, 2], f32)
        sel = A("sel", [64, 1], f32)
        d1 = A("d1", [64, 1], f32)
        idxf = A("idxf", [64, 1], f32)
        idxi = A("idxi", [64, 1], i32)

        # ---- load logits: block layout p<64 -> (row p, half0), p>=64 -> (row p-64, half1)
        nc.sync.dma_start(out=x[0:64, :], in_=lg_d[:, 0:H])
        nc.sync.dma_start(out=x[64:128, :], in_=lg_d[:, H:VP])
        nc.sync.dma_start(out=iot, in_=io_d)
        nc.sync.dma_start(out=cb8, in_=cb_d)
        # ---- membership scatter into pre-zeroed DRAM mask
        nc.sync.dma_start(out=pv, in_=pv_d)
        nc.sync.dma_start(out=rbs, in_=rb_d)
        nc.vector.memset(ones, 1)
        nc.vector.tensor_tensor(out=offs, in0=pv, in1=rbs.to_broadcast([128, T // 2]),
                                op=OP.add)
        for k in range(T // 2):
            nc.gpsimd.indirect_dma_start(
                out=mask_d, out_offset=bass.IndirectOffsetOnAxis(ap=offs[:, k:k + 1], axis=1),
                in_=ones, in_offset=None,
                bounds_check=RB * VP - 1, oob_is_err=False)
        nc.sync.dma_start(out=msk[0:64, :], in_=mask_d[:, 0:H])
        nc.sync.dma_start(out=msk[64:128, :], in_=mask_d[:, H:VP])
        # ---- penalty: x = mask ? min(x*rp, x/rp) : x   (chunked)
        for c in range(8):
            xc = x[:, c * CH:(c + 1) * CH]
            nc.scalar.mul(out=tA, in_=xc, mul=float(1.0 / rp))
            nc.vector.scalar_tensor_tensor(out=tB, in0=xc, scalar=float(rp),
                                           in1=tA, op0=OP.mult, op1=OP.min)
            nc.vector.copy_predicated(out=xc, mask=msk[:, c * CH:(c + 1) * CH], data=tB)
        # ---- row max (chunked)
        for c in range(8):
            nc.vector.tensor_reduce(out=zp[:, c:c + 1], in_=x[:, c * CH:(c + 1) * CH],
                                    axis=X, op=OP.max)
        nc.vector.tensor_reduce(out=rm, in_=zp, axis=X, op=OP.max)
        nc.sync.dma_start(out=s1_d[:, 0:1], in_=rm)
        nc.sync.dma_start(out=h2[:, 0:1], in_=s1_d[0:64, 0:1])
        nc.sync.dma_start(out=h2[:, 1:2], in_=s1_d[64:128, 0:1])
        nc.vector.tensor_reduce(out=m64, in_=h2, axis=X, op=OP.max)
        nc.vector.tensor_scalar_mul(negm64, m64, -1.0)
        nc.sync.dma_start(out=s1_d[0:64, 1:2], in_=negm64)
        nc.sync.dma_start(out=negm128[0:64, :], in_=s1_d[0:64, 1:2])
        nc.sync.dma_start(out=negm128[64:128, :], in_=s1_d[0:64, 1:2])
        # ---- Z = sum exp(x - m)
        for c in range(8):
            nc.scalar.activation(out=e, in_=x[:, c * CH:(c + 1) * CH], func=AF.Exp,
                                 bias=negm128, scale=1.0, accum_out=zp[:, c:c + 1])
        nc.vector.tensor_reduce(out=rm, in_=zp, axis=X, op=OP.add)
        nc.sync.dma_start(out=s1_d[:, 2:3], in_=rm)
        nc.sync.dma_start(out=z2[:, 0:1], in_=s1_d[0:64, 2:3])
        nc.sync.dma_start(out=z2[:, 1:2], in_=s1_d[64:128, 2:3])
        nc.vector.tensor_reduce(out=z64, in_=z2, axis=X, op=OP.add)
        nc.vector.tensor_scalar_mul(thrz, z64, float(top_p))
        # ---- candidates: top-8 per 786-subchunk
        for s in range(32):
            nc.vector.max(out=cand[:, 8 * s:8 * s + 8], in_=x[:, SUB * s:SUB * (s + 1)])
        nc.sync.dma_start(out=s2_d, in_=cand)
        nc.sync.dma_start(out=candr[:, 0:256], in_=s2_d[0:64, :])
        nc.sync.dma_start(out=candr[:, 256:512], in_=s2_d[64:128, :])
        # ---- sort top-56 of the 512 candidates (values only)
        nc.vector.tensor_copy(out=wk, in_=candr)
        for j in range(7):
            nc.vector.max(out=srt[:, 8 * j:8 * j + 8], in_=wk)
            nc.vector.match_replace(out=wk, in_to_replace=srt[:, 8 * j:8 * j + 8],
                                    in_values=wk, imm_value=-3e38)
        # ---- per-row small math
        nc.scalar.activation(out=e1[:, 0:56], in_=srt[:, 0:56], func=AF.Exp,
                             bias=negm64, scale=1.0)
        nc.vector.tensor_tensor_scan(out=Ssc[:, 0:56], data0=e1[:, 0:56], data1=e1[:, 0:56],
                                     initial=0.0, op0=OP.add, op1=OP.bypass)
        nc.vector.tensor_tensor(out=cle, in0=Ssc, in1=thrz.to_broadcast([64, 64]), op=OP.is_le)
        nc.vector.tensor_reduce(out=n1, in_=cle[:, 1:51], axis=X, op=OP.add)
        nc.vector.tensor_scalar(Jc, n1, 1.0, 50.0, op0=OP.add, op1=OP.min)
        nc.vector.tensor_scalar_add(Jm1, Jc, -1.0)
        nc.vector.tensor_tensor(out=oh, in0=iot, in1=Jm1.to_broadcast([64, 64]), op=OP.is_equal)
        nc.vector.tensor_tensor(out=tmp[:, 0:56], in0=srt[:, 0:56], in1=oh[:, 0:56], op=OP.mult)
        nc.vector.tensor_reduce(out=thr64, in_=tmp[:, 0:56], axis=X, op=OP.add)
        nc.vector.tensor_scalar_mul(negm64t, negm64, float(1.0 / temp))
        nc.scalar.activation(out=e2[:, 0:56], in_=srt[:, 0:56], func=AF.Exp,
                             bias=negm64t, scale=float(1.0 / temp))
        nc.vector.tensor_tensor(out=km, in0=iot, in1=Jc.to_broadcast([64, 64]), op=OP.is_lt)
        nc.vector.tensor_tensor(out=tmp[:, 0:56], in0=e2[:, 0:56], in1=km[:, 0:56], op=OP.mult)
        nc.vector.tensor_reduce(out=zj, in_=tmp[:, 0:56], axis=X, op=OP.add)
        nc.scalar.activation(out=lnzj, in_=zj, func=AF.Ln)
        nc.vector.tensor_scalar_mul(lnzj, lnzj, -1.0)
        nc.vector.tensor_tensor(out=b64, in0=lnzj, in1=negm64t, op=OP.add)
        nc.sync.dma_start(out=s1_d[0:64, 3:4], in_=thr64)
        nc.sync.dma_start(out=s1_d[0:64, 4:5], in_=b64)
        nc.sync.dma_start(out=thr128[0:64, :], in_=s1_d[0:64, 3:4])
        nc.sync.dma_start(out=thr128[64:128, :], in_=s1_d[0:64, 3:4])
        nc.sync.dma_start(out=b128[0:64, :], in_=s1_d[0:64, 4:5])
        nc.sync.dma_start(out=b128[64:128, :], in_=s1_d[0:64, 4:5])
        # ---- probs + gumbel argmax, chunked
        for c in range(8):
            xc = x[:, c * CH:(c + 1) * CH]
            nc.scalar.activation(out=e, in_=xc, func=AF.Exp, bias=b128,
                                 scale=float(1.0 / temp))
            nc.vector.tensor_tensor(out=tA, in0=xc, in1=thr128.to_broadcast([128, CH]),
                                    op=OP.is_ge)
            nc.vector.tensor_tensor(out=o, in0=tA, in1=e, op=OP.mult)
            nc.sync.dma_start(out=probs_d[:, c * CH:(c + 1) * CH], in_=o[0:64, :])
            if c < 7:
                nc.sync.dma_start(out=probs_d[:, H + c * CH:H + (c + 1) * CH],
                                  in_=o[64:128, :])
            else:
                nc.sync.dma_start(out=probs_d[:, H + c * CH:V],
                                  in_=o[64:128, 0:V - H - 7 * CH])
            nc.sync.dma_start(out=ru[0:64, :], in_=ru_d[:, c * CH:(c + 1) * CH])
            nc.sync.dma_start(out=ru[64:128, :], in_=ru_d[:, H + c * CH:H + (c + 1) * CH])
            nc.scalar.activation(out=rr, in_=ru, func=AF.Ln)
            nc.vector.reciprocal(out=ru, in_=rr)
            nc.vector.scalar_tensor_tensor(out=rr, in0=o, scalar=-1.0, in1=ru,
                                           op0=OP.mult, op1=OP.mult)
            nc.vector.max(out=mc[:, 8 * c:8 * c + 8], in_=rr)
            nc.vector.max_index(out=mi[:, 8 * c:8 * c + 8], in_max=mc[:, 8 * c:8 * c + 8],
                                in_values=rr)
        # ---- winner per partition, then per row
        nc.vector.max(out=sm8, in_=mc)
        for c in range(8):
            nc.vector.tensor_copy(out=pos[:, c:c + 1], in_=mc[:, 8 * c:8 * c + 1])
            nc.vector.tensor_copy(out=mif[:, c:c + 1], in_=mi[:, 8 * c:8 * c + 1])
        nc.vector.tensor_tensor(out=ohc, in0=pos,
                                in1=sm8[:, 0:1].to_broadcast([128, 8]), op=OP.is_equal)
        nc.vector.tensor_tensor(out=pos, in0=mif, in1=cb8, op=OP.add)
        nc.vector.tensor_tensor(out=pos, in0=pos, in1=ohc, op=OP.mult)
        nc.vector.tensor_reduce(out=posw, in_=pos, axis=X, op=OP.add)
        nc.sync.dma_start(out=s1_d[:, 5:6], in_=posw)
        nc.sync.dma_start(out=s1_d[:, 6:7], in_=sm8[:, 0:1])
        nc.sync.dma_start(out=wp2[:, 0:1], in_=s1_d[0:64, 5:6])
        nc.sync.dma_start(out=wp2[:, 1:2], in_=s1_d[64:128, 5:6])
        nc.sync.dma_start(out=wv2[:, 0:1], in_=s1_d[0:64, 6:7])
        nc.sync.dma_start(out=wv2[:, 1:2], in_=s1_d[64:128, 6:7])
        nc.vector.tensor_tensor(out=sel, in0=wv2[:, 0:1], in1=wv2[:, 1:2], op=OP.is_ge)
        nc.vector.tensor_tensor(out=d1, in0=wp2[:, 0:1], in1=wp2[:, 1:2], op=OP.subtract)
        nc.vector.tensor_scalar_add(d1, d1, float(-H))
        nc.vector.tensor_tensor(out=d1, in0=d1, in1=sel, op=OP.mult)
        nc.vector.tensor_tensor(out=idxf, in0=d1, in1=wp2[:, 1:2], op=OP.add)
        nc.vector.tensor_scalar_add(idxf, idxf, float(H))
        nc.vector.tensor_copy(out=idxi, in_=idxf)
        nc.sync.dma_start(out=idx_d, in_=idxi)

    nc.compile()
    return nc


def _get(temp, top_p, rp):
    key = (round(float(temp), 6), round(float(top_p), 6), round(float(rp), 6))
    if key not in _cache:
        _cache[key] = _build(float(temp), float(top_p), float(rp))
    return _cache[key]


def _make_runner(nc):
    import jax
    import numpy as np
    import concourse.mybir as mybir
    from concourse import bass2jax
    from concourse.bass2jax import _bass_exec_p, install_neuronx_cc_hook
    from jax.sharding import Mesh, PartitionSpec
    from jax.experimental.shard_map import shard_map

    install_neuronx_cc_hook()
    pname = nc.partition_id_tensor.name if nc.partition_id_tensor else None
    in_names, out_names, out_avals, zero_outs = [], [], [], []
    for alloc in nc.m.functions[0].allocations:
        if not isinstance(alloc, mybir.MemoryLocationSet):
            continue
        name = alloc.memorylocations[0].name
        if alloc.kind == "ExternalInput":
            if name != pname:
                in_names.append(name)
        elif alloc.kind == "ExternalOutput":
            out_names.append(name)
            shape = tuple(alloc.tensor_shape)
            dtype = mybir.dt.np(alloc.dtype)
            out_avals.append(jax.core.ShapedArray(shape, dtype))
            zero_outs.append(np.zeros(shape, dtype))
    n_params = len(in_names)
    n_outs = len(out_avals)
    all_in = list(in_names) + list(out_names)
    if pname is not None:
        all_in.append(pname)
    donate = tuple(range(n_params, n_params + n_outs))

    def _body(*args):
        ops = list(args)
        if pname is not None:
            ops.append(bass2jax.partition_id_tensor())
        outs = _bass_exec_p.bind(
            *ops, out_avals=tuple(out_avals), in_names=tuple(all_in),
            out_names=tuple(out_names), lowering_input_output_aliases=(),
            sim_require_finite=True, sim_require_nnan=True, nc=nc)
        return tuple(outs)

    devices = jax.devices()[:NC]
    mesh = Mesh(np.asarray(devices), ("core",))
    jitted = jax.jit(
        shard_map(_body, mesh=mesh, in_specs=(PartitionSpec("core"),) * (n_params + n_outs),
                  out_specs=(PartitionSpec("core"),) * n_outs, check_rep=False),
        donate_argnums=donate, keep_unused=True)

    def run(in_maps):
        per_core = [[np.asarray(m[n]) for n in in_names] for m in in_maps]
        concat_in = [np.concatenate([per_core[c][i] for c in range(NC)], axis=0)
                     for i in range(n_params)]
        concat_zeros = [np.zeros((NC * z.shape[0], *z.shape[1:]), z.dtype)
                        for z in zero_outs]
        outs = jitted(*concat_in, *concat_zeros)
        return [
            {name: np.asarray(outs[i]).reshape(NC, *out_avals[i].shape)[c]
             for i, name in enumerate(out_names)}
            for c in range(NC)
        ]
    return run


def kernel(logits, previous_tokens, rand_u, temperature, top_p, repetition_penalty):
    from concourse import bass_utils
    logits = np.asarray(logits, np.float32)
    prev = np.asarray(previous_tokens)
    randu = np.asarray(rand_u, np.float32)
    temp = float(np.asarray(temperature).ravel()[0])
    tp = float(np.asarray(top_p).ravel()[0])
    rp = float(np.asarray(repetition_penalty).ravel()[0])
    key = (round(temp, 6), round(tp, 6), round(rp, 6), "run")
    if key not in _cache:
        _cache[key] = _make_runner(_get(temp, tp, rp))
    runner = _cache[key]

    lg = np.full((B, VP), NEG, np.float32)
    lg[:, :V] = logits
    ru = np.full((B, VP), 0.5, np.float32)
    ru[:, :V] = randu
    pv128 = prev.astype(np.int32).reshape(B // RB, RB, 2, T // 2).transpose(0, 2, 1, 3)
    rb = (np.arange(128, dtype=np.int32) % 64 * VP).reshape(128, 1)
    io = np.broadcast_to(np.arange(64, dtype=np.float32), (64, 64)).copy()
    cb = np.broadcast_to((np.arange(8, dtype=np.float32) * CH), (128, 8)).copy()

    in_maps = []
    for c in range(NC):
        in_maps.append({
            "lg": lg[c * RB:(c + 1) * RB],
            "ru": ru[c * RB:(c + 1) * RB],
            "pv": np.ascontiguousarray(pv128[c].reshape(128, T // 2)),
            "rb": rb, "io": io, "cb": cb,
        })
    results = runner(in_maps)
    probs = np.concatenate([results[c]["probs"] for c in range(NC)], axis=0)
    idx = np.concatenate([results[c]["idx"] for c in range(NC)], axis=0)
    out_dtype = np.int64 if prev.dtype == np.int64 else np.int32
    return idx.astype(out_dtype), probs
